# revision 33
# baseline (speedup 1.0000x reference)
"""Distance-based attention (nn_Attention_67989332296336) on 8 TRN2 NeuronCores.

Math per batch element b (S=1024, E=H=A=256):
    d2[t,j]  = |x_t|^2 + |x_j|^2 - 2 x_t.x_j
    dist     = sqrt(max(d2,0)+eps)
    scores   = w_sim*dist + b_sim
    A        = softmax_j(scores)
    G        = A @ h
    Z        = tanh([G, h] @ W_g^T + b_g)

Sharding: batch dim B=32 split over 8 cores (4 per core), weights replicated.

Per-core strategy (v3: dist symmetry + fp8 DoubleRow gram/PV):
  - x/h loaded via SWDGE casting DMAs (f32 -> bf16).  x^T built on
    TensorE (bf16 PSUM) and cast to fp8e4 on the DVE evac; h^T for two
    batches via the sync-ring DMA-xbar, the other two on TensorE.
  - gram only for the upper block-triangle as fp8e4 DoubleRow matmuls
    (K=256/instr); "-0.5|x_j|^2" via a bf16 K=8 block-diag aug matmul;
    |x_t|^2+MARGIN is the sqrt bias (scale=-2).  sqrt runs per t-tile
    on the upper cols only; lower dist blocks are TensorE transposes of
    upper ones (dist is symmetric) evacuated by the DVE.
  - exp runs full-width in 4-row chunks, writing P straight to fp8e4;
    PV is DoubleRow fp8 (P pairs x hw1 pairs), with a ones column in
    hw1 giving softmax denominators.  Z = tanh(PV/den + hW2 + bg).
  - hW = h @ [W1|W2]^T per batch; the W1 half is cast to fp8 (DVE and
    ScalarE alternating), the W2 half to bf16 on the DVE; bg added via
    a K=1 aug matmul.
  - ScalarE table discipline: all Sqrt precede all Exp/Tanh.
  - a ~4us dummy-matmul burst trips the PE HAM clock gate under the
    initial DMAs.
"""

import sys

import numpy as np

if "/opt/trn_rl_repo" not in sys.path:
    sys.path.append("/opt/trn_rl_repo")

import concourse.bacc as bacc
import concourse.bass as bass
import concourse.mybir as mybir
import concourse.tile as tile
from concourse.bass import ts
from concourse.bass_utils import run_bass_kernel_spmd
from concourse.masks import make_identity

F32 = mybir.dt.float32
BF16 = mybir.dt.bfloat16
FP8 = mybir.dt.float8e4
AF = mybir.ActivationFunctionType
OP = mybir.AluOpType
PM = mybir.MatmulPerfMode

S = 1024
B = 32
NCORES = 8
BS = B // NCORES  # batches per core
E = 256
H = 256
A = 256
NT = S // 128  # 8 t-tiles
MARGIN = 12.0  # replaces max(d2,0)+eps; absorbs fp8 gram rounding (~cancels in softmax)
HW1P = 272  # hw1 pitch (257 used, padded to %16 for DoubleRow pair strides


def build_graph():
    nc = bacc.Bacc("TRN2", target_bir_lowering=False, debug=False)

    x_ext = nc.declare_dram_parameter("x", [BS, S, E], F32, isOutput=False)
    h_ext = nc.declare_dram_parameter("h", [BS, S, H], F32, isOutput=False)
    w_ext = nc.declare_dram_parameter("w_sim", [1, 1], F32, isOutput=False)
    wg_ext = nc.declare_dram_parameter("W_g", [A, 2 * H], F32, isOutput=False)
    bg_ext = nc.declare_dram_parameter("b_g", [1, A], F32, isOutput=False)
    out_ext = nc.declare_dram_parameter("out", [BS, S, A], F32, isOutput=True)

    with tile.TileContext(nc) as tc:
        with (
            tc.tile_pool(name="consts", bufs=1) as consts,
            tc.tile_pool(name="dist", bufs=BS) as distp,
            tc.tile_pool(name="work", bufs=2) as work,
            tc.tile_pool(name="nat", bufs=4) as natp,
            tc.tile_pool(name="small", bufs=2) as smallp,
            tc.tile_pool(name="zt", bufs=2) as ztp,
            tc.tile_pool(name="hTp", bufs=4) as hTp,
            tc.tile_pool(name="hwp", bufs=4) as hwp,
            tc.tile_pool(name="pP", bufs=4) as pP,
            tc.tile_pool(name="ps_bigb", bufs=2, space="PSUM") as psbb,
            tc.tile_pool(name="ps_d2", bufs=2, space="PSUM") as psd,
            tc.tile_pool(name="ps_f32", bufs=2, space="PSUM") as psf,
        ):
            # PE HAM warm-up burst under the initial DMAs
            warm_in = consts.tile([128, 128], BF16)
            nc.vector.memset(warm_in, 1.0)
            warm_ps = psf.tile([128, 512], F32, tag="big")
            for _ in range(44):
                nc.tensor.matmul(
                    warm_ps[:, 0:128], warm_in[:], warm_in[:], start=True, stop=True
                )

            # prefetch inputs; order chosen so phase-1 (x) and the sync-ring
            # h-transposes (h0/h1) are fed as early as possible:
            # x0, h0, x1, h1, x2, x3, h2, h3
            xnat_list = [None] * BS
            hnat_list = [None] * BS
            def load_x(b):
                xnat = natp.tile([128, NT, E], BF16, tag="nat")
                xnat_list[b] = xnat
                nc.gpsimd.dma_start(
                    out=xnat,
                    in_=x_ext[b].rearrange("(p i) e -> p i e", p=128),
                )
            def load_h(b):
                hnat = natp.tile([128, NT, H], BF16, tag="nat")
                hnat_list[b] = hnat
                nc.gpsimd.dma_start(
                    out=hnat,
                    in_=h_ext[b].rearrange("(p i) e -> p i e", p=128),
                )
            load_x(0); load_x(1); load_x(2); load_x(3)
            load_h(0); load_h(1); load_h(2); load_h(3)

            # ---------------- constants ----------------
            ident = consts.tile([128, 128], F32)
            make_identity(nc, ident)
            identb = consts.tile([128, 128], BF16)
            nc.vector.tensor_copy(identb, ident)
            ones_stage = consts.tile([8, 128], F32)
            nc.vector.memset(ones_stage, 1.0)
            ones_row = consts.tile([1, 128], BF16)
            nc.vector.tensor_copy(ones_row, ones_stage[0:1, :])
            ones8 = consts.tile([8, 128], BF16)
            nc.vector.tensor_copy(ones8, ones_stage)
            zero_stage = consts.tile([8, S], F32)
            nc.vector.memset(zero_stage, 0.0)

            w_col = consts.tile([128, 1], F32)
            nc.sync.dma_start(out=w_col, in_=w_ext[:].partition_broadcast(128))
            bg_stage = consts.tile([1, A], F32)
            nc.sync.dma_start(out=bg_stage, in_=bg_ext[:])
            bg_row = consts.tile([1, A], BF16)
            nc.vector.tensor_copy(bg_row, bg_stage)

            wnat = consts.tile([128, 2, 2 * H], F32)
            nc.sync.dma_start(
                out=wnat, in_=wg_ext[:].rearrange("(m p) k -> p m k", m=2)
            )
            w12t = consts.tile([128, 2, 2 * H], BF16)

            # ---------------- phase 1: upper-triangle distances ----------------
            # Gram bursts are paired (b0+b1, b2+b3) and pinned dense so the
            # HAM clock-gate flips to 8/8 and stays there through each pair.
            d_tiles = []
            xT_list = []
            sqrt_instrs = []
            gram_last = [None] * BS
            phase1_dve_last = None
            xt_evacs = []
            prev_evac = [None]
            exp0_instrs = []
            for b in range(BS):
                xT8 = work.tile([128, 2, S], FP8, tag="xT")
                xT_list.append(xT8)
                sqmcol = smallp.tile([128, NT], F32, tag="sqm")
                biasp = smallp.tile([128, NT], F32, tag="bias")
                blockdiag = smallp.tile([8, S], BF16, tag="bd")
                bi = nc.vector.tensor_copy(blockdiag, zero_stage)
                if prev_evac[0] is not None:
                    tile.add_dep_helper(
                        bi.ins, prev_evac[0].ins, sync=False,
                        reason="dve-bd-after-prev-evac",
                    )
                d_b = distp.tile([128, NT, S], BF16, tag="D")
                d_tiles.append(d_b)

                xnat = xnat_list[b]

                # |x_t|^2 chain first: it feeds the aug matmuls via the
                # block-diag scatter and must not trail the x^T evacs
                for i in range(NT):
                    scr = smallp.tile([128, E], F32, tag="scr")
                    sm = nc.vector.scalar_tensor_tensor(
                        out=scr,
                        in0=xnat[:, i, :],
                        scalar=1.0,
                        in1=xnat[:, i, :],
                        op0=OP.mult,
                        op1=OP.mult,
                        accum_out=sqmcol[:, i : i + 1],
                    )
                    if prev_evac[0] is not None:
                        tile.add_dep_helper(
                            sm.ins, prev_evac[0].ins, sync=False,
                            reason="dve-sqm-after-prev-evac",
                        )
                pb = nc.vector.tensor_scalar_add(out=biasp, in0=sqmcol, scalar1=MARGIN)
                phase1_dve_last = pb
                sqmb = smallp.tile([128, NT], BF16, tag="sqmb")
                nc.vector.tensor_copy(sqmb[:], sqmcol[:])
                sq8 = psbb.tile([8, 128], BF16, tag="bigb")
                nc.tensor.transpose(sq8[:], sqmb[:], identb[:])
                sq8sb = smallp.tile([8, 128], BF16, tag="sq8sb")
                nc.vector.tensor_scalar_mul(sq8sb[:], sq8[:], -0.5)
                bd = blockdiag[:]
                diag_view = bass.AP(
                    tensor=bd.tensor, offset=bd.offset, ap=[[S + 128, NT], [1, 128]]
                )
                nc.gpsimd.dma_start(out=diag_view, in_=sq8sb[:])

                # x^T: TensorE transpose pairs into bf16 PSUM + DVE evac
                for p2 in range(NT // 2):
                    i0, i1 = 2 * p2, 2 * p2 + 1
                    ps = psbb.tile([128, 512], BF16, tag="bigb")
                    for t2, i in enumerate((i0, i1)):
                        for k2 in range(2):
                            ti = nc.tensor.transpose(
                                ps[:, t2 * 256 + k2 * 128 : t2 * 256 + (k2 + 1) * 128],
                                xnat[:, i, ts(k2, 128)],
                                identb[:],
                            )
                            if b >= 2 and gram_last[b - 2] is not None:
                                tile.add_dep_helper(
                                    ti.ins, gram_last[b - 2].ins, sync=False,
                                    reason="pe-xt-after-pairprev-gram",
                                )
                    ei = nc.vector.tensor_copy(
                        xT8[:, :, i0 * 128 : i0 * 128 + 256].rearrange(
                            "p k (t f) -> p t k f", t=2
                        ),
                        ps[:].rearrange("p (t k f) -> p t k f", t=2, k=2),
                    )
                    xt_evacs.append(ei)
                prev_evac[0] = xt_evacs[-1]

                for i in range(NT):
                    lo = 128 * i
                    d2 = psd.tile([128, S], F32, tag="d2")
                    chunks = [(lo, 512), (512, S)] if lo < 512 else [(lo, S)]
                    for c0, c1 in chunks:
                        nc.tensor.matmul(
                            d2[:, c0:c1],
                            xT8[:, :, lo : lo + 128],
                            xT8[:, :, c0:c1],
                            start=True,
                            stop=False,
                            perf_mode=PM.DoubleRow,
                        )
                        mi = nc.tensor.matmul(
                            d2[:, c0:c1],
                            ones8[:],
                            blockdiag[:, c0:c1],
                            start=False,
                            stop=True,
                        )
                        gram_last[b] = mi
                    # dist = sqrt(-2*psum + |x_t|^2 + MARGIN), one per row
                    si = nc.scalar.activation(
                        out=d_b[:, i, lo:S],
                        in_=d2[:, lo:S],
                        func=AF.Sqrt,
                        bias=biasp[:, i : i + 1],
                        scale=-2.0,
                    )
                    sqrt_instrs.append(si)

            # W_g (A, 2H) -> W12T transposes; pinned after b0's grams so a
            # slow W_g DMA cannot head-block the PE queue at startup
            for k2 in range(2):
                ps = psf.tile([128, 512], F32, tag="big")
                for w in range(2):
                    for m in range(2):
                        ti = nc.tensor.transpose(
                            ps[:, w * 256 + m * 128 : w * 256 + (m + 1) * 128],
                            wnat[:, m, w * 256 + k2 * 128 : w * 256 + (k2 + 1) * 128],
                            ident[:],
                        )
                        tile.add_dep_helper(
                            ti.ins, gram_last[0].ins, sync=False,
                            reason="pe-w12t-after-b0-gram",
                        )
                nc.vector.tensor_copy(w12t[:, k2, :], ps[:])

            # ------- per batch: dist-lower + h^T transposes -------
            hT_list = []
            hw1_list = []
            hw2_list = []
            for b in range(BS):
                d_b = d_tiles[b]
                # lower dist blocks = transposes of upper (dist is symmetric)
                for i in range(NT - 1):
                    nb = NT - 1 - i
                    tp = psbb.tile([128, 1024], BF16, tag="bigb")
                    for u in range(nb):
                        ti = nc.tensor.transpose(
                            tp[:, u * 128 : (u + 1) * 128],
                            d_b[:, i, (i + 1 + u) * 128 : (i + 2 + u) * 128],
                            identb[:],
                        )
                        tile.add_dep_helper(
                            ti.ins, gram_last[b].ins,
                            sync=False, reason="pe-dT-after-grams",
                        )
                    ci = nc.vector.tensor_copy(
                        d_b[:, i + 1 :, ts(i, 128)],
                        tp[:, 0 : nb * 128].rearrange("p (u f) -> p u f", u=nb),
                    )
                    tile.add_dep_helper(
                        ci.ins, phase1_dve_last.ins, sync=False,
                        reason="dve-dT-evac-after-phase1",
                    )

                hT = hTp.tile([128, 2, S], BF16, tag="hT")
                hT_list.append(hT)
                hnat = hnat_list[b]
                for p2 in range(NT // 2):
                    i0, i1 = 2 * p2, 2 * p2 + 1
                    ps = psbb.tile([128, 512], BF16, tag="bigb")
                    for t2, i in enumerate((i0, i1)):
                        for k2 in range(2):
                            ti = nc.tensor.transpose(
                                ps[:, t2 * 256 + k2 * 128 : t2 * 256 + (k2 + 1) * 128],
                                hnat[:, i, ts(k2, 128)],
                                identb[:],
                            )
                            tile.add_dep_helper(
                                ti.ins, gram_last[b].ins,
                                sync=False, reason="pe-hT-after-grams",
                            )
                    ci = nc.vector.tensor_copy(
                        hT[:, :, i0 * 128 : i0 * 128 + 256].rearrange(
                            "p k (t f) -> p t k f", t=2
                        ),
                        ps[:].rearrange("p (t k f) -> p t k f", t=2, k=2),
                    )
                    tile.add_dep_helper(
                        ci.ins, phase1_dve_last.ins, sync=False,
                        reason="dve-hT-evac-after-phase1",
                    )
                # hW = h @ [W1|W2]^T (+ bg on the W2 half)
                hT = hT_list[b]
                hw1 = hwp.tile([128, NT, HW1P], FP8, tag="hw1")
                hw2 = hwp.tile([128, NT, A], BF16, tag="hw2")
                hw1_list.append(hw1)
                hw2_list.append(hw2)
                for m in range(NT):
                    ps = psf.tile([128, 512], F32, tag="big")
                    nc.tensor.matmul(
                        ps[:], hT[:, 0, ts(m, 128)], w12t[:, 0, :],
                        start=True, stop=False,
                    )
                    nc.tensor.matmul(
                        ps[:], hT[:, 1, ts(m, 128)], w12t[:, 1, :],
                        start=False, stop=False,
                    )
                    nc.tensor.matmul(
                        ps[:, 256:512], ones_row[:], bg_row[:],
                        start=False, stop=True,
                    )
                    # W1 half -> fp8 (alternate DVE/ScalarE), W2 half -> bf16
                    if m % 2 == 0:
                        nc.vector.tensor_copy(hw1[:, m, 0:256], ps[:, 0:256])
                    else:
                        # copy is in every ACT table set: safe to weave into
                        # the sqrt era; deps are on-chip so no head-block risk
                        nc.scalar.copy(hw1[:, m, 0:256], ps[:, 0:256])
                    nc.vector.tensor_copy(hw2[:, m, :], ps[:, 256:512])
                nc.vector.memset(hw1[:, :, 256:257], 1.0)


            # ------- softmax + PV + gate per batch -------
            for b in range(BS):
                d_b = d_tiles[b]
                hw1, hw2 = hw1_list[b], hw2_list[b]
                # P = exp(w * dist), full rows, straight to fp8.
                # Table-era split: exp-b0 only waits for b0/b1 sqrts; the
                # b2/b3 sqrts then wait for exp-b0 (2 extra table loads buy
                # a full overlap of b0's phase 2 with b2/b3's phase 1).
                p8 = pP.tile([128, NT, S], FP8, tag="P")
                sq_dep = sqrt_instrs
                for i4 in range(0, NT, 4):
                    ei = nc.scalar.activation(
                        out=p8[:, i4 : i4 + 4, :],
                        in_=d_b[:, i4 : i4 + 4, :],
                        func=AF.Exp,
                        scale=w_col[:, 0:1],
                    )
                    if b == 0:
                        exp0_instrs.append(ei)
                    for si in sq_dep:
                        tile.add_dep_helper(
                            ei.ins, si.ins, sync=False, reason="act-table-order"
                        )

                for i4 in range(0, NT, 4):
                    zs = ztp.tile([128, 4, A], F32, tag="zs")
                    for u in range(4):
                        i = i4 + u
                        pv = psbb.tile([128, 512], F32, tag="bigb")
                        for k in range(2):
                            nc.tensor.matmul(
                                pv[:, 0 : A + 1],
                                p8[:, k, ts(i, 128)],
                                hw1[:, k, 0 : A + 1],
                                start=(k == 0),
                                stop=False,
                            )
                        for k2 in range(2, NT, 2):
                            nc.tensor.matmul(
                                pv[:, 0 : A + 1],
                                p8[:, k2 : k2 + 2, ts(i, 128)],
                                hw1[:, k2 : k2 + 2, 0 : A + 1],
                                start=False,
                                stop=(k2 == NT - 2),
                                perf_mode=PM.DoubleRow,
                            )
                        rp_i = smallp.tile([128, 1], F32, tag="rp_i")
                        nc.vector.reciprocal(rp_i[:], pv[:, A : A + 1])
                        nc.vector.scalar_tensor_tensor(
                            out=zs[:, u, :],
                            in0=pv[:, 0:A],
                            scalar=rp_i[:, 0:1],
                            in1=hw2[:, i, :],
                            op0=OP.mult,
                            op1=OP.add,
                        )
                    zo = ztp.tile([128, 4, A], F32, tag="zo")
                    thi = nc.scalar.activation(
                        out=zo[:].rearrange("p a b -> p (a b)"),
                        in_=zs[:].rearrange("p a b -> p (a b)"),
                        func=AF.Tanh,
                    )
                    for si in sqrt_instrs:
                        tile.add_dep_helper(
                            thi.ins, si.ins, sync=False, reason="act-table-order"
                        )
                    nc.gpsimd.dma_start(
                        out=out_ext[b].rearrange("(p i) a -> p i a", p=128)[
                            :, i4 : i4 + 4, :
                        ],
                        in_=zo,
                    )

    nc.compile()
    return nc


_CACHED = {}


def _get_graph():
    if "nc" not in _CACHED:
        _CACHED["nc"] = build_graph()
    return _CACHED["nc"]


def _run(inputs, trace=False, **kw):
    nc = _get_graph()
    x = np.asarray(inputs["x"], dtype=np.float32)
    h = np.asarray(inputs["h"], dtype=np.float32)
    w_sim = np.asarray(inputs["w_sim"], dtype=np.float32).reshape(1, 1)
    W_g = np.ascontiguousarray(np.asarray(inputs["W_g"], dtype=np.float32))
    b_g = np.asarray(inputs["b_g"], dtype=np.float32).reshape(1, A)
    in_maps = []
    for c in range(NCORES):
        in_maps.append(
            {
                "x": np.ascontiguousarray(
                    x[:, c * BS : (c + 1) * BS, :].transpose(1, 0, 2)
                ),
                "h": np.ascontiguousarray(
                    h[:, c * BS : (c + 1) * BS, :].transpose(1, 0, 2)
                ),
                "w_sim": w_sim,
                "W_g": W_g,
                "b_g": b_g,
            }
        )
    res = run_bass_kernel_spmd(nc, in_maps, list(range(NCORES)), trace=trace, **kw)
    out = np.concatenate(
        [res.results[c]["out"].transpose(1, 0, 2) for c in range(NCORES)], axis=1
    )
    return out, res


def kernel(**inputs):
    out, _ = _run(inputs, trace=False)
    return out


if __name__ == "__main__":
    rng = np.random.default_rng(0)
    ins = {
        "x": rng.standard_normal((S, B, E), dtype=np.float32),
        "h": rng.standard_normal((S, B, E), dtype=np.float32),
        "w_sim": np.array([0.03], dtype=np.float32),
        "b_sim": np.array([0.01], dtype=np.float32),
        "W_g": (rng.standard_normal((A, 2 * H)) * 0.05).astype(np.float32),
        "b_g": np.zeros(A, dtype=np.float32),
    }
    out = kernel(**ins)
    print("out", out.shape, out.dtype, np.abs(out).mean())
